# revision 30
# baseline (speedup 1.0000x reference)
"""Causal attention (B=1, H=16, S=4096, D=64, f32) on 8 trn2 NeuronCores.

Strategy (head-parallel, 2 heads per core):
  - Host pre-transposes Q, K per head to [D, S] (d-major) so the QK^T
    matmul needs no on-device transpose: S^T[k, q] = sum_d K^T[d,k] Q^T[d,q].
  - S^T layout keeps k on PSUM partitions and q on the free axis, so
    exp(S^T) -> P^T lands in SBUF exactly as the rhs of the PV matmul:
    O^T[d, q] = sum_k V[k, d] P^T[k, q], accumulated over k-tiles in PSUM.
  - l[q] = sum_k exp is obtained for free by appending a ones column to V
    (column 64 of the PV matmul output). Host epilogue: O = (O^T[:64]/l).T.

Hybrid exp across TWO engines (the single biggest win over v1):
  Host scales q,k by sqrt((2^10/ln2)/8) each, so the QK^T matmul directly
  produces y = (2^10/ln2) * (q.k/8) in PSUM. Then either engine can
  finish softmax's exp:
   - ScalarE: activation(Exp, scale=ln2/2^10) recovers exact exp(q.k/8).
   - VectorE: tensor_scalar add of B = 15*2^10 - C (C=55) with int16
     output; the int16 bit pattern REINTERPRETED as float16 is
     Schraudolph's fast-exp approximation (~3% sawtooth error, which
     softmax normalization cancels to ~2e-3 in the final output). One
     1x DVE op per element, comparable throughput to ScalarE's exp.
  Off-diagonal chunks are routed greedily to balance the two engines;
  diagonal chunks always take the exact ScalarE path (short softmax rows
  are most error-sensitive). Engine busy measured on HW: PE ~121-125us,
  ScalarE ~105-111us, VectorE ~91-98us; wall ~147-149us with PE gaps
  down to ~8-9us and ~10us NEFF preamble.

Pipeline (the second big win): chunks of exactly 2 k-tiles, stp PSUM
triple-buffered (3x2 banks + 2 otp = 8), and PV emission LAGGED TWO
chunks behind its QK/exp (PE FIFO per iteration: QK(i)...PV(i-2)), so
every PE instruction's dependencies resolve ~2 chunks of PE work before
the FIFO reaches it. This took the wall from ~183us to ~147us.

Known dead ends (measured, do not retry blindly):
  - PE row-group pairing of the two QK matmuls is only partially
    concurrent (qk avg 199ns vs 213 ideal); PV matmuls pay ~85ns each of
    exposed LDWEIGHTS+drain because their 128-row V weights conflict
    with any in-flight matmul and walrus runs with --enable-ldw-opt=false
    (no background weight buffer).
  - Padding V to 128 cols for FWL: no effect on the PV overhead.
  - Splitting exp per 512-tile, cross-engine exp splits, interleaving
    head0-ascending/head1-descending chunks (worse even on the lag-2
    pipeline), GpSimd corner masks (~3x slower than DVE), warmup matmul
    count: all neutral-to-worse.
  - fp8 DoubleRow / DoubleRowSwInterleave PV (2 k-tiles per matmul,
    weights must be full 128-col pairs to pass s3_lw_valid_num_active_cols):
    compiles and CoreSim passes (~8-10e-3 err) and runs ~142us, but HW
    returns NaN -- hardware fp8/interleave semantics do not match CoreSim.

  Causality: k-tiles strictly below the diagonal are skipped entirely; the
  4 diagonal k-tiles per q-block keep only q-columns >= 128*t (QK and PV
  run with a reduced moving dim; PSUM bank-clear zeroes the rest), and the
  single 128x128 triangular corner is masked by an in-place VectorE
  multiply with a constant 0/1 tile.

Matmul dtypes: fp16 throughout (q,k pre-scaled on host; V cast host-side;
P^T is either ScalarE fp16 exp output or the int16 Schraudolph bit
pattern viewed as fp16; the exp trick uses fp16's 2^10 mantissa scale). QK^T matmuls go two-at-a-time in disjoint PE row
groups (rows 0-63 / 64-127 hold identical data).
"""

import os
import sys
import numpy as np

sys.path.insert(0, "/opt/trn_rl_repo")

import concourse.bass as bass
import concourse.mybir as mybir
from concourse.tile import TileContext

B, H, S, D = 1, 16, 4096, 64

PROGRAM_META: dict[str, str] = {}   # instruction name -> kind (for tracing)


def _note(inst, kind):
    try:
        inst.annotate(kind)
        PROGRAM_META[str(inst.ins.name)] = kind
    except Exception:
        pass

N_CORES = 8
H_PER = H // N_CORES          # heads per core
QB = 512                      # q-block (matmul moving dim / PSUM bank)
KT = 128                      # k-tile (contraction tile for PV matmul)
NQB = S // QB                 # 8
NKT = S // KT                 # 32
VW = D + 1                    # V columns + ones column for the l sum

F32 = mybir.dt.float32
F32R = mybir.dt.float32r
F16 = mybir.dt.float16
I16 = mybir.dt.int16

EXP_A = float(2.0 ** 10) / float(np.log(2.0))   # y = EXP_A * (q.k/8)
SCHRAUDOLPH_C = 55.0
SCHRAUDOLPH_B = 15.0 * 2.0 ** 10 - SCHRAUDOLPH_C
QK_SIDE_SCALE = float(np.sqrt(EXP_A / 8.0))     # folded into q AND k


def round_fp32r(x: np.ndarray) -> np.ndarray:
    """fp32 -> fp32r: round-half-to-even at mantissa bit 12 (keep 11 bits)."""
    u = np.ascontiguousarray(x, dtype=np.float32).view(np.uint32)
    r = (u + np.uint32(0x7FF) + ((u >> np.uint32(12)) & np.uint32(1))) & np.uint32(
        0xFFFFF000
    )
    return r.view(np.float32)


def build_program() -> bass.Bass:
    nc = bass.Bass()
    # qk rows 0-63 and 64-127 hold identical qT|kT data: the duplicate lets
    # two QK^T matmuls run concurrently in disjoint PE row groups
    qk_d = nc.declare_dram_parameter("qk", [H_PER, 2 * D, 2 * S], F16, isOutput=False)
    va_d = nc.declare_dram_parameter("va", [H_PER, 128, NKT * VW], F16, isOutput=False)
    mk_d = nc.declare_dram_parameter("mk", [128, KT], F16, isOutput=False)
    oT_d = nc.declare_dram_parameter("outT", [H_PER, VW, S], F32, isOutput=True)

    with TileContext(nc) as tc:
        with (
            tc.tile_pool(name="const", bufs=1) as cpool,
            tc.tile_pool(name="io", bufs=1) as iopool,
            tc.tile_pool(name="pt", bufs=int(os.environ.get("ATTN_PTB", "6"))) as ppool,
            tc.tile_pool(name="st", bufs=3, space="PSUM") as stpool,
            tc.tile_pool(name="ot", bufs=2, space="PSUM") as otpool,
        ):
            # single 128x128 0/1 lower-triangular corner mask (keep qq >= kk)
            ctri = cpool.tile([128, KT], F16, name="ctri")
            nc.sync.dma_start(out=ctri, in_=mk_d[:, :])

            # warmup matmuls: ~4us of sustained matmul activity moves the PE
            # clock (HAM) 1.2 -> 2.4 GHz before real compute. Uses the ctri
            # tile (first DMA to land) so they start immediately, no memset.
            n_warm = int(os.environ.get("ATTN_WARM", "36"))
            if n_warm:
                wps = otpool.tile([128, KT], F32, name="warmps", tag="otp")
                for _ in range(n_warm):
                    mi = nc.tensor.matmul(
                        out=wps, lhsT=ctri, rhs=ctri,
                        start=True, stop=True,
                    )
                    _note(mi, "warm")

            head_ctx = []
            for h in range(H_PER):
                vas = iopool.tile([128, NKT * VW], F16, name=f"vas{h}")
                qkts = iopool.tile([2 * D, 2 * S], F16, name=f"qkts{h}")
                outs = iopool.tile([VW, S], F32, name=f"outs{h}")
                # q-block 0 only needs the first 512 columns of q/k and the
                # first 4 V k-tiles: stage those first so compute starts
                # while the bulk still streams in
                if h == 0:
                    nc.sync.dma_start(out=vas[:, 0:4 * VW], in_=va_d[h][:, 0:4 * VW])
                    nc.sync.dma_start(out=qkts[:, 0:QB], in_=qk_d[h][:, 0:QB])
                    nc.sync.dma_start(
                        out=qkts[:, S:S + QB], in_=qk_d[h][:, S:S + QB]
                    )
                    nc.sync.dma_start(
                        out=vas[:, 4 * VW:], in_=va_d[h][:, 4 * VW:]
                    )
                    nc.sync.dma_start(out=qkts[:, QB:S], in_=qk_d[h][:, QB:S])
                    nc.sync.dma_start(
                        out=qkts[:, S + QB:2 * S], in_=qk_d[h][:, S + QB:2 * S]
                    )
                else:
                    nc.sync.dma_start(out=vas, in_=va_d[h])
                    # split halves onto separate DMA queues
                    nc.sync.dma_start(out=qkts[:, 0:S], in_=qk_d[h][:, 0:S])
                    nc.sync.dma_start(
                        out=qkts[:, S:2 * S], in_=qk_d[h][:, S:2 * S]
                    )
                head_ctx.append((vas, qkts, outs))

            # flat chunk list over (head, q-block): chunks of <=3 k-tiles;
            # one 3-bank PSUM tile + one exp (ScalarE or VectorE) per chunk
            def head_chunks(h, js):
                # chunks of exactly 2 k-tiles: one QK row-group pair, one
                # 2-bank stp tile -> stp can triple-buffer (3x2+2 = 8 banks)
                out = []
                for j in js:
                    n_kt = 4 * (j + 1)          # causal: k-tiles 0..4j+3
                    for k0 in range(0, n_kt, 2):
                        out.append((h, j, k0, 2, n_kt))
                return out

            # interleave head 0 (ascending j) with head 1 (descending j) so
            # the PE always has a large chunk in flight while the other
            # stream is in a small/diagonal region (smooths startup + drain)
            if os.environ.get("ATTN_ILV", "1") == "1":
                s0 = head_chunks(0, range(NQB))
                s1 = head_chunks(1, range(NQB - 1, -1, -1))
                all_chunks = []
                for a, b in zip(s0, s1):
                    all_chunks.append(a)
                    all_chunks.append(b)
            else:
                all_chunks = head_chunks(0, range(NQB)) + head_chunks(
                    1, range(NQB)
                )

            # engine routing: diagonal chunks -> ScalarE (exact exp);
            # off-diagonal chunks balance ScalarE/VectorE busy-time with a
            # preference for alternation (keeps both engines concurrently
            # busy within the software pipeline).
            # routing[idx] = list of (col_lo, col_hi, engine) exp pieces.
            # Diagonal chunks: pure ScalarE (exact exp where softmax rows are
            # short/peaked). Off-diagonal: first tile on one engine, rest on
            # the other, alternating; both engines then work the same chunk
            # concurrently, halving the exp latency ahead of the PV matmuls.
            eng_ns = {"act": 0.0, "dve": 0.0}

            dvec = float(os.environ.get("ATTN_DVEC", "145"))

            def exp_cost(eng, fd):
                return (fd + 222.0) / 1.2 if eng == "act" else (fd + dvec) / 0.96

            routing = []
            flip = False
            for (h, j, k0, clen, n_kt) in all_chunks:
                is_diag = (k0 + clen - 1) >= 4 * j
                pieces = []
                if is_diag:
                    pieces.append((0, clen * QB, "act"))
                    n_corner = sum(
                        1 for u in range(clen) if (k0 + u) - 4 * j >= 0
                    )
                    eng_ns["dve"] += n_corner * 260.0
                elif clen == 1:
                    eng = "act" if eng_ns["act"] <= eng_ns["dve"] else "dve"
                    pieces.append((0, QB, eng))
                else:
                    e1 = "dve" if flip else "act"
                    e2 = "act" if flip else "dve"
                    flip = not flip
                    if eng_ns[e1] > eng_ns[e2] + 4000.0:
                        e1, e2 = e2, e1
                    pieces.append((0, QB, e1))
                    pieces.append((QB, clen * QB, e2))
                for lo, hi, eng in pieces:
                    eng_ns[eng] += exp_cost(eng, hi - lo)
                if k0 + clen == n_kt:
                    eng_ns["act"] += 570.0       # PSUM->SBUF out copy
                routing.append(pieces)

            otp_box = {}

            def emit_qks(idx):
                h, j, k0, clen, n_kt = all_chunks[idx]
                vas, qkts, outs = head_ctx[h]
                stp = stpool.tile([128, 2 * QB], F32, name="stp", tag="stp")
                # QK^T matmuls two-at-a-time in disjoint row groups
                # (rows 0-63 / 64-127 hold identical q,k data) so the PE
                # runs them concurrently. Diagonal tiles only produce
                # q-columns >= 128t (start=True bank-clear zeroes the rest).
                u = 0
                while u < clen:
                    for r in range(2 if u + 1 < clen else 1):
                        ki = k0 + u + r
                        t = ki - 4 * j
                        off = KT * t if t > 0 else 0
                        row = slice(r * D, (r + 1) * D)
                        mi = nc.tensor.matmul(
                            out=stp[:, (u + r) * QB + off:(u + r + 1) * QB],
                            lhsT=qkts[row, S + ki * KT:S + (ki + 1) * KT],
                            rhs=qkts[row, j * QB + off:(j + 1) * QB],
                            start=True,
                            stop=True,
                        )
                        _note(mi, "qk_diag" if t > 0 else "qk")
                    u += 2 if u + 1 < clen else 1
                pt = ppool.tile([128, 2 * QB], F16, name="pt", tag="pt")
                return stp, pt

            def emit_exp(idx, qk_pt):
                h, j, k0, clen, n_kt = all_chunks[idx]
                stp, pt = qk_pt
                # valid (written) column runs: diagonal tiles only produced
                # q-columns >= 128t, so merge per-tile valid ranges into
                # contiguous runs and exp only those (PSUM outside them is
                # uninitialized)
                runs = []
                for u in range(clen):
                    t = (k0 + u) - 4 * j
                    off = KT * t if t > 0 else 0
                    lo, hi = u * QB + off, (u + 1) * QB
                    if runs and runs[-1][1] == lo:
                        runs[-1][1] = hi
                    else:
                        runs.append([lo, hi])
                for plo, phi, eng in routing[idx]:
                    for rlo, rhi in runs:
                        lo, hi = max(plo, rlo), min(phi, rhi)
                        if lo >= hi:
                            continue
                        if eng == "act":
                            nc.scalar.activation(
                                out=pt[:, lo:hi], in_=stp[:, lo:hi],
                                func=mybir.ActivationFunctionType.Exp,
                                scale=1.0 / EXP_A,
                            )
                        else:
                            nc.vector.tensor_scalar(
                                out=pt[:, lo:hi].bitcast(I16),
                                in0=stp[:, lo:hi],
                                scalar1=SCHRAUDOLPH_B,
                                scalar2=None,
                                op0=mybir.AluOpType.add,
                            )

            def emit_masks(idx, qk_pt):
                # in-place 128x128 triangular corner masks (VectorE). Emitted
                # BEFORE the next chunk's VectorE convert so diagonal PVs are
                # not head-of-line blocked behind a 1.7us convert.
                h, j, k0, clen, n_kt = all_chunks[idx]
                stp, pt = qk_pt
                for u in range(clen):
                    t = (k0 + u) - 4 * j
                    if t >= 0:
                        cs = u * QB + KT * t
                        mask_eng = (
                            nc.gpsimd
                            if os.environ.get("ATTN_GMASK", "0") == "1"
                            else nc.vector
                        )
                        mask_eng.tensor_mul(
                            out=pt[:, cs:cs + KT],
                            in0=pt[:, cs:cs + KT],
                            in1=ctri,
                        )

            def emit_pvs(idx, qk_pt):
                h, j, k0, clen, n_kt = all_chunks[idx]
                stp, pt = qk_pt
                vas, qkts, outs = head_ctx[h]
                if (h, j) not in otp_box:
                    otp_box[(h, j)] = otpool.tile(
                        [VW, QB], F32, name="otp", tag="otp"
                    )
                otp = otp_box[(h, j)]
                for u in range(clen):
                    ki = k0 + u
                    t = ki - 4 * j
                    off = KT * t if t >= 0 else 0
                    mi = nc.tensor.matmul(
                        out=otp[:, off:QB],
                        lhsT=vas[:, ki * VW:(ki + 1) * VW],
                        rhs=pt[:, u * QB + off:(u + 1) * QB],
                        start=(ki == 0),
                        stop=(ki == n_kt - 1),
                    )
                    _note(mi, "pv_diag" if t >= 0 else "pv")
                if k0 + clen == n_kt:       # last chunk of this q-block
                    nc.scalar.copy(
                        out=outs[:, j * QB:(j + 1) * QB], in_=otp
                    )
                    nc.sync.dma_start(
                        out=oT_d[h][:, j * QB:(j + 1) * QB],
                        in_=outs[:, j * QB:(j + 1) * QB],
                    )

            # 2-deep software pipeline. Per-engine FIFO orders per iteration:
            #   PE:  QK(i) ... PV(i-2)    (PV deps resolved ~2 chunks early)
            #   DVE: mask(i-1), conv(i)   (masks not HOL-blocked by convert)
            #   ACT: exp(i)
            LAG = int(os.environ.get("ATTN_LAG", "3"))
            hist = {}
            n_chunks = len(all_chunks)
            for idx in range(n_chunks):
                hist[idx] = emit_qks(idx)
                if idx >= 1:
                    emit_masks(idx - 1, hist[idx - 1])
                emit_exp(idx, hist[idx])
                if idx >= LAG:
                    emit_pvs(idx - LAG, hist.pop(idx - LAG))
            emit_masks(n_chunks - 1, hist[n_chunks - 1])
            for idx in range(n_chunks - LAG, n_chunks):
                emit_pvs(idx, hist.pop(idx))

    # TRN2 allows at most 1 semaphore wait per instruction (the fp32r
    # matmul's LDWEIGHTS slot enforces it); split surplus waits into
    # standalone EventSemaphore instructions like the bacc flow does.
    import concourse.bacc as baccmod

    baccmod._bass_rust.generate_event_semaphores(nc)
    return nc


_PROGRAM_CACHE: dict[str, bass.Bass] = {}


def get_program() -> bass.Bass:
    if "p" not in _PROGRAM_CACHE:
        _PROGRAM_CACHE["p"] = build_program()
    return _PROGRAM_CACHE["p"]


def make_corner_mask() -> np.ndarray:
    kk = np.arange(128)[:, None]
    qq = np.arange(KT)[None, :]
    return np.ascontiguousarray((qq >= kk).astype(np.float16))


def make_in_maps(q, k, v):
    q = np.asarray(q, dtype=np.float32)
    k = np.asarray(k, dtype=np.float32)
    v = np.asarray(v, dtype=np.float32)
    mk = make_corner_mask()
    in_maps = []
    for c in range(N_CORES):
        hs = [H_PER * c + i for i in range(H_PER)]
        qk = np.empty((H_PER, 2 * D, 2 * S), dtype=np.float16)
        va = np.empty((H_PER, 128, NKT, VW), dtype=np.float16)
        for i, h in enumerate(hs):
            qk[i, 0:D, 0:S] = q[0, h].T * QK_SIDE_SCALE
            qk[i, 0:D, S:2 * S] = k[0, h].T * QK_SIDE_SCALE
            qk[i, D:2 * D, :] = qk[i, 0:D, :]
            # [S, D] -> k-tiles on partitions: [128, NKT, D]
            va[i, :, :, :D] = v[0, h].reshape(NKT, KT, D).transpose(1, 0, 2)
            va[i, :, :, D] = 1.0
        in_maps.append(
            {
                "qk": qk,
                "va": np.ascontiguousarray(va.reshape(H_PER, 128, NKT * VW)),
                "mk": mk,
            }
        )
    return in_maps


def assemble_output(results) -> np.ndarray:
    out = np.empty((B, H, S, D), dtype=np.float32)
    for c in range(N_CORES):
        oT = results[c]["outT"]  # [H_PER, VW, S]
        for i in range(H_PER):
            h = H_PER * c + i
            out[0, h] = (oT[i, :D, :] / oT[i, D:D + 1, :]).T
    return out


def run_sharded(q, k, v, trace: bool = False):
    from concourse.bass_utils import run_bass_kernel_spmd

    nc = get_program()
    in_maps = make_in_maps(q, k, v)
    res = run_bass_kernel_spmd(
        nc, in_maps, list(range(N_CORES)), trace=trace
    )
    return assemble_output(res.results), res


def kernel(q, k, v, mask=None) -> np.ndarray:
    # mask is deterministically the causal tril mask; causality is baked in.
    out, _ = run_sharded(q, k, v, trace=False)
    return out


# revision 31
# speedup vs baseline: 1.0218x; 1.0218x over previous
"""Causal attention (B=1, H=16, S=4096, D=64, f32) on 8 trn2 NeuronCores.

Strategy (head-parallel, 2 heads per core):
  - Host pre-transposes Q, K per head to [D, S] (d-major) so the QK^T
    matmul needs no on-device transpose: S^T[k, q] = sum_d K^T[d,k] Q^T[d,q].
  - S^T layout keeps k on PSUM partitions and q on the free axis, so
    exp(S^T) -> P^T lands in SBUF exactly as the rhs of the PV matmul:
    O^T[d, q] = sum_k V[k, d] P^T[k, q], accumulated over k-tiles in PSUM.
  - l[q] = sum_k exp is obtained for free by appending a ones column to V
    (column 64 of the PV matmul output). Host epilogue: O = (O^T[:64]/l).T.

Hybrid exp across TWO engines (the single biggest win over v1):
  Host scales q,k by sqrt((2^10/ln2)/8) each, so the QK^T matmul directly
  produces y = (2^10/ln2) * (q.k/8) in PSUM. Then either engine can
  finish softmax's exp:
   - ScalarE: activation(Exp, scale=ln2/2^10) recovers exact exp(q.k/8).
   - VectorE: tensor_scalar add of B = 15*2^10 - C (C=55) with int16
     output; the int16 bit pattern REINTERPRETED as float16 is
     Schraudolph's fast-exp approximation (~3% sawtooth error, which
     softmax normalization cancels to ~2e-3 in the final output). One
     1x DVE op per element, comparable throughput to ScalarE's exp.
  Off-diagonal chunks are routed greedily to balance the two engines;
  diagonal chunks always take the exact ScalarE path (short softmax rows
  are most error-sensitive). Engine busy measured on HW: PE ~136us,
  ScalarE ~108us, VectorE ~90us -> the PE is the bottleneck; wall
  ~183us of which ~10us NEFF preamble and ~32us PE dependency gaps.

Known dead ends (measured, do not retry blindly):
  - PE row-group pairing of the two QK matmuls is only partially
    concurrent (qk avg 199ns vs 213 ideal); PV matmuls pay ~85ns each of
    exposed LDWEIGHTS+drain because their 128-row V weights conflict
    with any in-flight matmul and walrus runs with --enable-ldw-opt=false
    (no background weight buffer).
  - Padding V to 128 cols for FWL: no effect on the PV overhead.
  - Splitting exp per 512-tile, cross-engine exp splits, interleaving
    head0-ascending/head1-descending chunks, GpSimd corner masks
    (~3x slower than DVE), warmup matmul count: all neutral-to-worse.

  Causality: k-tiles strictly below the diagonal are skipped entirely; the
  4 diagonal k-tiles per q-block keep only q-columns >= 128*t (QK and PV
  run with a reduced moving dim; PSUM bank-clear zeroes the rest), and the
  single 128x128 triangular corner is masked by an in-place VectorE
  multiply with a constant 0/1 tile.

Matmul dtypes: fp16 throughout (q,k pre-scaled on host; V cast host-side;
P^T is either ScalarE fp16 exp output or the int16 Schraudolph bit
pattern viewed as fp16; the exp trick uses fp16's 2^10 mantissa scale). QK^T matmuls go two-at-a-time in disjoint PE row
groups (rows 0-63 / 64-127 hold identical data).
"""

import os
import sys
import numpy as np

sys.path.insert(0, "/opt/trn_rl_repo")

import concourse.bass as bass
import concourse.mybir as mybir
from concourse.tile import TileContext

B, H, S, D = 1, 16, 4096, 64

PROGRAM_META: dict[str, str] = {}   # instruction name -> kind (for tracing)


def _note(inst, kind):
    try:
        inst.annotate(kind)
        PROGRAM_META[str(inst.ins.name)] = kind
    except Exception:
        pass

N_CORES = 8
H_PER = H // N_CORES          # heads per core
QB = 512                      # q-block (matmul moving dim / PSUM bank)
KT = 128                      # k-tile (contraction tile for PV matmul)
NQB = S // QB                 # 8
NKT = S // KT                 # 32
VW = D + 1                    # V columns + ones column for the l sum

F32 = mybir.dt.float32
F32R = mybir.dt.float32r
F16 = mybir.dt.float16
I16 = mybir.dt.int16

EXP_A = float(2.0 ** 10) / float(np.log(2.0))   # y = EXP_A * (q.k/8)
SCHRAUDOLPH_C = 55.0
SCHRAUDOLPH_B = 15.0 * 2.0 ** 10 - SCHRAUDOLPH_C
QK_SIDE_SCALE = float(np.sqrt(EXP_A / 8.0))     # folded into q AND k


def round_fp32r(x: np.ndarray) -> np.ndarray:
    """fp32 -> fp32r: round-half-to-even at mantissa bit 12 (keep 11 bits)."""
    u = np.ascontiguousarray(x, dtype=np.float32).view(np.uint32)
    r = (u + np.uint32(0x7FF) + ((u >> np.uint32(12)) & np.uint32(1))) & np.uint32(
        0xFFFFF000
    )
    return r.view(np.float32)


def build_program() -> bass.Bass:
    nc = bass.Bass()
    # qk rows 0-63 and 64-127 hold identical qT|kT data: the duplicate lets
    # two QK^T matmuls run concurrently in disjoint PE row groups
    qk_d = nc.declare_dram_parameter("qk", [H_PER, 2 * D, 2 * S], F16, isOutput=False)
    va_d = nc.declare_dram_parameter("va", [H_PER, 128, NKT * VW], F16, isOutput=False)
    mk_d = nc.declare_dram_parameter("mk", [128, KT], F16, isOutput=False)
    oT_d = nc.declare_dram_parameter("outT", [H_PER, VW, S], F32, isOutput=True)

    with TileContext(nc) as tc:
        with (
            tc.tile_pool(name="const", bufs=1) as cpool,
            tc.tile_pool(name="io", bufs=1) as iopool,
            tc.tile_pool(name="pt", bufs=int(os.environ.get("ATTN_PTB", "4"))) as ppool,
            tc.tile_pool(name="st", bufs=3, space="PSUM") as stpool,
            tc.tile_pool(name="ot", bufs=2, space="PSUM") as otpool,
        ):
            # single 128x128 0/1 lower-triangular corner mask (keep qq >= kk)
            ctri = cpool.tile([128, KT], F16, name="ctri")
            nc.sync.dma_start(out=ctri, in_=mk_d[:, :])

            # warmup matmuls: ~4us of sustained matmul activity moves the PE
            # clock (HAM) 1.2 -> 2.4 GHz before real compute. Uses the ctri
            # tile (first DMA to land) so they start immediately, no memset.
            n_warm = int(os.environ.get("ATTN_WARM", "0"))
            if n_warm:
                wps = otpool.tile([128, KT], F32, name="warmps", tag="otp")
                for _ in range(n_warm):
                    mi = nc.tensor.matmul(
                        out=wps, lhsT=ctri, rhs=ctri,
                        start=True, stop=True,
                    )
                    _note(mi, "warm")

            head_ctx = []
            for h in range(H_PER):
                vas = iopool.tile([128, NKT * VW], F16, name=f"vas{h}")
                qkts = iopool.tile([2 * D, 2 * S], F16, name=f"qkts{h}")
                outs = iopool.tile([VW, S], F32, name=f"outs{h}")
                # q-block 0 only needs the first 512 columns of q/k and the
                # first 4 V k-tiles: stage those first so compute starts
                # while the bulk still streams in
                if h == 0:
                    nc.sync.dma_start(out=vas[:, 0:4 * VW], in_=va_d[h][:, 0:4 * VW])
                    nc.sync.dma_start(out=qkts[:, 0:QB], in_=qk_d[h][:, 0:QB])
                    nc.sync.dma_start(
                        out=qkts[:, S:S + QB], in_=qk_d[h][:, S:S + QB]
                    )
                    nc.sync.dma_start(
                        out=vas[:, 4 * VW:], in_=va_d[h][:, 4 * VW:]
                    )
                    nc.sync.dma_start(out=qkts[:, QB:S], in_=qk_d[h][:, QB:S])
                    nc.sync.dma_start(
                        out=qkts[:, S + QB:2 * S], in_=qk_d[h][:, S + QB:2 * S]
                    )
                else:
                    nc.sync.dma_start(out=vas, in_=va_d[h])
                    # split halves onto separate DMA queues
                    nc.sync.dma_start(out=qkts[:, 0:S], in_=qk_d[h][:, 0:S])
                    nc.sync.dma_start(
                        out=qkts[:, S:2 * S], in_=qk_d[h][:, S:2 * S]
                    )
                head_ctx.append((vas, qkts, outs))

            # flat chunk list over (head, q-block): chunks of <=3 k-tiles;
            # one 3-bank PSUM tile + one exp (ScalarE or VectorE) per chunk
            def head_chunks(h, js):
                # chunks of exactly 2 k-tiles: one QK row-group pair, one
                # 2-bank stp tile -> stp can triple-buffer (3x2+2 = 8 banks)
                out = []
                for j in js:
                    n_kt = 4 * (j + 1)          # causal: k-tiles 0..4j+3
                    for k0 in range(0, n_kt, 2):
                        out.append((h, j, k0, 2, n_kt))
                return out

            # interleave head 0 (ascending j) with head 1 (descending j) so
            # the PE always has a large chunk in flight while the other
            # stream is in a small/diagonal region (smooths startup + drain)
            if os.environ.get("ATTN_ILV", "1") == "1":
                s0 = head_chunks(0, range(NQB))
                s1 = head_chunks(1, range(NQB - 1, -1, -1))
                all_chunks = []
                for a, b in zip(s0, s1):
                    all_chunks.append(a)
                    all_chunks.append(b)
            else:
                all_chunks = head_chunks(0, range(NQB)) + head_chunks(
                    1, range(NQB)
                )

            # engine routing: diagonal chunks -> ScalarE (exact exp);
            # off-diagonal chunks balance ScalarE/VectorE busy-time with a
            # preference for alternation (keeps both engines concurrently
            # busy within the software pipeline).
            # routing[idx] = list of (col_lo, col_hi, engine) exp pieces.
            # Diagonal chunks: pure ScalarE (exact exp where softmax rows are
            # short/peaked). Off-diagonal: first tile on one engine, rest on
            # the other, alternating; both engines then work the same chunk
            # concurrently, halving the exp latency ahead of the PV matmuls.
            eng_ns = {"act": 0.0, "dve": 0.0}

            dvec = float(os.environ.get("ATTN_DVEC", "145"))

            def exp_cost(eng, fd):
                return (fd + 222.0) / 1.2 if eng == "act" else (fd + dvec) / 0.96

            routing = []
            flip = False
            for (h, j, k0, clen, n_kt) in all_chunks:
                is_diag = (k0 + clen - 1) >= 4 * j
                pieces = []
                if is_diag:
                    pieces.append((0, clen * QB, "act"))
                    n_corner = sum(
                        1 for u in range(clen) if (k0 + u) - 4 * j >= 0
                    )
                    eng_ns["dve"] += n_corner * 260.0
                elif clen == 1:
                    eng = "act" if eng_ns["act"] <= eng_ns["dve"] else "dve"
                    pieces.append((0, QB, eng))
                else:
                    e1 = "dve" if flip else "act"
                    e2 = "act" if flip else "dve"
                    flip = not flip
                    if eng_ns[e1] > eng_ns[e2] + 4000.0:
                        e1, e2 = e2, e1
                    pieces.append((0, QB, e1))
                    pieces.append((QB, clen * QB, e2))
                for lo, hi, eng in pieces:
                    eng_ns[eng] += exp_cost(eng, hi - lo)
                if k0 + clen == n_kt:
                    eng_ns["dve"] += 754.0       # PSUM->SBUF out copy
                routing.append(pieces)

            otp_box = {}

            def emit_qks(idx):
                h, j, k0, clen, n_kt = all_chunks[idx]
                vas, qkts, outs = head_ctx[h]
                stp = stpool.tile([128, 2 * QB], F32, name="stp", tag="stp")
                # QK^T matmuls two-at-a-time in disjoint row groups
                # (rows 0-63 / 64-127 hold identical q,k data) so the PE
                # runs them concurrently. Diagonal tiles only produce
                # q-columns >= 128t (start=True bank-clear zeroes the rest).
                u = 0
                while u < clen:
                    for r in range(2 if u + 1 < clen else 1):
                        ki = k0 + u + r
                        t = ki - 4 * j
                        off = KT * t if t > 0 else 0
                        row = slice(r * D, (r + 1) * D)
                        mi = nc.tensor.matmul(
                            out=stp[:, (u + r) * QB + off:(u + r + 1) * QB],
                            lhsT=qkts[row, S + ki * KT:S + (ki + 1) * KT],
                            rhs=qkts[row, j * QB + off:(j + 1) * QB],
                            start=True,
                            stop=True,
                        )
                        _note(mi, "qk_diag" if t > 0 else "qk")
                    u += 2 if u + 1 < clen else 1
                pt = ppool.tile([128, 2 * QB], F16, name="pt", tag="pt")
                return stp, pt

            def emit_exp(idx, qk_pt):
                h, j, k0, clen, n_kt = all_chunks[idx]
                stp, pt = qk_pt
                # valid (written) column runs: diagonal tiles only produced
                # q-columns >= 128t, so merge per-tile valid ranges into
                # contiguous runs and exp only those (PSUM outside them is
                # uninitialized)
                runs = []
                for u in range(clen):
                    t = (k0 + u) - 4 * j
                    off = KT * t if t > 0 else 0
                    lo, hi = u * QB + off, (u + 1) * QB
                    if runs and runs[-1][1] == lo:
                        runs[-1][1] = hi
                    else:
                        runs.append([lo, hi])
                for plo, phi, eng in routing[idx]:
                    for rlo, rhi in runs:
                        lo, hi = max(plo, rlo), min(phi, rhi)
                        if lo >= hi:
                            continue
                        if eng == "act":
                            nc.scalar.activation(
                                out=pt[:, lo:hi], in_=stp[:, lo:hi],
                                func=mybir.ActivationFunctionType.Exp,
                                scale=1.0 / EXP_A,
                            )
                        else:
                            nc.vector.tensor_scalar(
                                out=pt[:, lo:hi].bitcast(I16),
                                in0=stp[:, lo:hi],
                                scalar1=SCHRAUDOLPH_B,
                                scalar2=None,
                                op0=mybir.AluOpType.add,
                            )

            def emit_masks(idx, qk_pt):
                # in-place 128x128 triangular corner masks (VectorE). Emitted
                # BEFORE the next chunk's VectorE convert so diagonal PVs are
                # not head-of-line blocked behind a 1.7us convert.
                h, j, k0, clen, n_kt = all_chunks[idx]
                stp, pt = qk_pt
                for u in range(clen):
                    t = (k0 + u) - 4 * j
                    if t >= 0:
                        cs = u * QB + KT * t
                        mask_eng = (
                            nc.gpsimd
                            if os.environ.get("ATTN_GMASK", "0") == "1"
                            else nc.vector
                        )
                        mask_eng.tensor_mul(
                            out=pt[:, cs:cs + KT],
                            in0=pt[:, cs:cs + KT],
                            in1=ctri,
                        )

            def emit_pvs(idx, qk_pt):
                h, j, k0, clen, n_kt = all_chunks[idx]
                stp, pt = qk_pt
                vas, qkts, outs = head_ctx[h]
                if (h, j) not in otp_box:
                    otp_box[(h, j)] = otpool.tile(
                        [VW, QB], F32, name="otp", tag="otp"
                    )
                otp = otp_box[(h, j)]
                for u in range(clen):
                    ki = k0 + u
                    t = ki - 4 * j
                    off = KT * t if t >= 0 else 0
                    mi = nc.tensor.matmul(
                        out=otp[:, off:QB],
                        lhsT=vas[:, ki * VW:(ki + 1) * VW],
                        rhs=pt[:, u * QB + off:(u + 1) * QB],
                        start=(ki == 0),
                        stop=(ki == n_kt - 1),
                    )
                    _note(mi, "pv_diag" if t >= 0 else "pv")
                if k0 + clen == n_kt:       # last chunk of this q-block
                    nc.vector.tensor_copy(
                        out=outs[:, j * QB:(j + 1) * QB], in_=otp
                    )
                    nc.sync.dma_start(
                        out=oT_d[h][:, j * QB:(j + 1) * QB],
                        in_=outs[:, j * QB:(j + 1) * QB],
                    )

            # 2-deep software pipeline. Per-engine FIFO orders per iteration:
            #   PE:  QK(i) ... PV(i-2)    (PV deps resolved ~2 chunks early)
            #   DVE: mask(i-1), conv(i)   (masks not HOL-blocked by convert)
            #   ACT: exp(i)
            hist = {}
            n_chunks = len(all_chunks)
            for idx in range(n_chunks):
                hist[idx] = emit_qks(idx)
                if idx >= 1:
                    emit_masks(idx - 1, hist[idx - 1])
                emit_exp(idx, hist[idx])
                if idx >= 2:
                    emit_pvs(idx - 2, hist.pop(idx - 2))
            emit_masks(n_chunks - 1, hist[n_chunks - 1])
            emit_pvs(n_chunks - 2, hist.pop(n_chunks - 2))
            emit_pvs(n_chunks - 1, hist.pop(n_chunks - 1))

    # TRN2 allows at most 1 semaphore wait per instruction (the fp32r
    # matmul's LDWEIGHTS slot enforces it); split surplus waits into
    # standalone EventSemaphore instructions like the bacc flow does.
    import concourse.bacc as baccmod

    baccmod._bass_rust.generate_event_semaphores(nc)
    return nc


_PROGRAM_CACHE: dict[str, bass.Bass] = {}


def get_program() -> bass.Bass:
    if "p" not in _PROGRAM_CACHE:
        _PROGRAM_CACHE["p"] = build_program()
    return _PROGRAM_CACHE["p"]


def make_corner_mask() -> np.ndarray:
    kk = np.arange(128)[:, None]
    qq = np.arange(KT)[None, :]
    return np.ascontiguousarray((qq >= kk).astype(np.float16))


def make_in_maps(q, k, v):
    q = np.asarray(q, dtype=np.float32)
    k = np.asarray(k, dtype=np.float32)
    v = np.asarray(v, dtype=np.float32)
    mk = make_corner_mask()
    in_maps = []
    for c in range(N_CORES):
        hs = [H_PER * c + i for i in range(H_PER)]
        qk = np.empty((H_PER, 2 * D, 2 * S), dtype=np.float16)
        va = np.empty((H_PER, 128, NKT, VW), dtype=np.float16)
        for i, h in enumerate(hs):
            qk[i, 0:D, 0:S] = q[0, h].T * QK_SIDE_SCALE
            qk[i, 0:D, S:2 * S] = k[0, h].T * QK_SIDE_SCALE
            qk[i, D:2 * D, :] = qk[i, 0:D, :]
            # [S, D] -> k-tiles on partitions: [128, NKT, D]
            va[i, :, :, :D] = v[0, h].reshape(NKT, KT, D).transpose(1, 0, 2)
            va[i, :, :, D] = 1.0
        in_maps.append(
            {
                "qk": qk,
                "va": np.ascontiguousarray(va.reshape(H_PER, 128, NKT * VW)),
                "mk": mk,
            }
        )
    return in_maps


def assemble_output(results) -> np.ndarray:
    out = np.empty((B, H, S, D), dtype=np.float32)
    for c in range(N_CORES):
        oT = results[c]["outT"]  # [H_PER, VW, S]
        for i in range(H_PER):
            h = H_PER * c + i
            out[0, h] = (oT[i, :D, :] / oT[i, D:D + 1, :]).T
    return out


def run_sharded(q, k, v, trace: bool = False):
    from concourse.bass_utils import run_bass_kernel_spmd

    nc = get_program()
    in_maps = make_in_maps(q, k, v)
    res = run_bass_kernel_spmd(
        nc, in_maps, list(range(N_CORES)), trace=trace
    )
    return assemble_output(res.results), res


def kernel(q, k, v, mask=None) -> np.ndarray:
    # mask is deterministically the causal tril mask; causality is baked in.
    out, _ = run_sharded(q, k, v, trace=False)
    return out


# revision 32
# speedup vs baseline: 1.0265x; 1.0045x over previous
"""Causal attention (B=1, H=16, S=4096, D=64, f32) on 8 trn2 NeuronCores.

Strategy (head-parallel, 2 heads per core):
  - Host pre-transposes Q, K per head to [D, S] (d-major) so the QK^T
    matmul needs no on-device transpose: S^T[k, q] = sum_d K^T[d,k] Q^T[d,q].
  - S^T layout keeps k on PSUM partitions and q on the free axis, so
    exp(S^T) -> P^T lands in SBUF exactly as the rhs of the PV matmul:
    O^T[d, q] = sum_k V[k, d] P^T[k, q], accumulated over k-tiles in PSUM.
  - l[q] = sum_k exp is obtained for free by appending a ones column to V
    (column 64 of the PV matmul output). Host epilogue: O = (O^T[:64]/l).T.

Hybrid exp across TWO engines (the single biggest win over v1):
  Host scales q,k by sqrt((2^10/ln2)/8) each, so the QK^T matmul directly
  produces y = (2^10/ln2) * (q.k/8) in PSUM. Then either engine can
  finish softmax's exp:
   - ScalarE: activation(Exp, scale=ln2/2^10) recovers exact exp(q.k/8).
   - VectorE: tensor_scalar add of B = 15*2^10 - C (C=55) with int16
     output; the int16 bit pattern REINTERPRETED as float16 is
     Schraudolph's fast-exp approximation (~3% sawtooth error, which
     softmax normalization cancels to ~2e-3 in the final output). One
     1x DVE op per element, comparable throughput to ScalarE's exp.
  Off-diagonal chunks are routed greedily to balance the two engines;
  diagonal chunks always take the exact ScalarE path (short softmax rows
  are most error-sensitive). Engine busy measured on HW: PE ~136us,
  ScalarE ~108us, VectorE ~90us -> the PE is the bottleneck; wall
  ~183us of which ~10us NEFF preamble and ~32us PE dependency gaps.

Known dead ends (measured, do not retry blindly):
  - PE row-group pairing of the two QK matmuls is only partially
    concurrent (qk avg 199ns vs 213 ideal); PV matmuls pay ~85ns each of
    exposed LDWEIGHTS+drain because their 128-row V weights conflict
    with any in-flight matmul and walrus runs with --enable-ldw-opt=false
    (no background weight buffer).
  - Padding V to 128 cols for FWL: no effect on the PV overhead.
  - Splitting exp per 512-tile, cross-engine exp splits, interleaving
    head0-ascending/head1-descending chunks, GpSimd corner masks
    (~3x slower than DVE), warmup matmul count: all neutral-to-worse.

  Causality: k-tiles strictly below the diagonal are skipped entirely; the
  4 diagonal k-tiles per q-block keep only q-columns >= 128*t (QK and PV
  run with a reduced moving dim; PSUM bank-clear zeroes the rest), and the
  single 128x128 triangular corner is masked by an in-place VectorE
  multiply with a constant 0/1 tile.

Matmul dtypes: fp16 throughout (q,k pre-scaled on host; V cast host-side;
P^T is either ScalarE fp16 exp output or the int16 Schraudolph bit
pattern viewed as fp16; the exp trick uses fp16's 2^10 mantissa scale). QK^T matmuls go two-at-a-time in disjoint PE row
groups (rows 0-63 / 64-127 hold identical data).
"""

import os
import sys
import numpy as np

sys.path.insert(0, "/opt/trn_rl_repo")

import concourse.bass as bass
import concourse.mybir as mybir
from concourse.tile import TileContext

B, H, S, D = 1, 16, 4096, 64

PROGRAM_META: dict[str, str] = {}   # instruction name -> kind (for tracing)


def _note(inst, kind):
    try:
        inst.annotate(kind)
        PROGRAM_META[str(inst.ins.name)] = kind
    except Exception:
        pass

N_CORES = 8
H_PER = H // N_CORES          # heads per core
QB = 512                      # q-block (matmul moving dim / PSUM bank)
KT = 128                      # k-tile (contraction tile for PV matmul)
NQB = S // QB                 # 8
NKT = S // KT                 # 32
VW = D + 1                    # V columns + ones column for the l sum

F32 = mybir.dt.float32
F32R = mybir.dt.float32r
F16 = mybir.dt.float16
I16 = mybir.dt.int16

EXP_A = float(2.0 ** 10) / float(np.log(2.0))   # y = EXP_A * (q.k/8)
SCHRAUDOLPH_C = 55.0
SCHRAUDOLPH_B = 15.0 * 2.0 ** 10 - SCHRAUDOLPH_C
QK_SIDE_SCALE = float(np.sqrt(EXP_A / 8.0))     # folded into q AND k


def round_fp32r(x: np.ndarray) -> np.ndarray:
    """fp32 -> fp32r: round-half-to-even at mantissa bit 12 (keep 11 bits)."""
    u = np.ascontiguousarray(x, dtype=np.float32).view(np.uint32)
    r = (u + np.uint32(0x7FF) + ((u >> np.uint32(12)) & np.uint32(1))) & np.uint32(
        0xFFFFF000
    )
    return r.view(np.float32)


def build_program() -> bass.Bass:
    nc = bass.Bass()
    # qk rows 0-63 and 64-127 hold identical qT|kT data: the duplicate lets
    # two QK^T matmuls run concurrently in disjoint PE row groups
    qk_d = nc.declare_dram_parameter("qk", [H_PER, 2 * D, 2 * S], F16, isOutput=False)
    va_d = nc.declare_dram_parameter("va", [H_PER, 128, NKT * VW], F16, isOutput=False)
    mk_d = nc.declare_dram_parameter("mk", [128, KT], F16, isOutput=False)
    oT_d = nc.declare_dram_parameter("outT", [H_PER, VW, S], F32, isOutput=True)

    with TileContext(nc) as tc:
        with (
            tc.tile_pool(name="const", bufs=1) as cpool,
            tc.tile_pool(name="io", bufs=1) as iopool,
            tc.tile_pool(name="pt", bufs=int(os.environ.get("ATTN_PTB", "4"))) as ppool,
            tc.tile_pool(name="st", bufs=3, space="PSUM") as stpool,
            tc.tile_pool(name="ot", bufs=2, space="PSUM") as otpool,
        ):
            # single 128x128 0/1 lower-triangular corner mask (keep qq >= kk)
            ctri = cpool.tile([128, KT], F16, name="ctri")
            nc.sync.dma_start(out=ctri, in_=mk_d[:, :])

            # warmup matmuls: ~4us of sustained matmul activity moves the PE
            # clock (HAM) 1.2 -> 2.4 GHz before real compute. Uses the ctri
            # tile (first DMA to land) so they start immediately, no memset.
            n_warm = int(os.environ.get("ATTN_WARM", "0"))
            if n_warm:
                wps = otpool.tile([128, KT], F32, name="warmps", tag="otp")
                for _ in range(n_warm):
                    mi = nc.tensor.matmul(
                        out=wps, lhsT=ctri, rhs=ctri,
                        start=True, stop=True,
                    )
                    _note(mi, "warm")

            head_ctx = []
            for h in range(H_PER):
                vas = iopool.tile([128, NKT * VW], F16, name=f"vas{h}")
                qkts = iopool.tile([2 * D, 2 * S], F16, name=f"qkts{h}")
                outs = iopool.tile([VW, S], F32, name=f"outs{h}")
                # q-block 0 only needs the first 512 columns of q/k and the
                # first 4 V k-tiles: stage those first so compute starts
                # while the bulk still streams in
                if h == 0:
                    # stage the first two q-blocks' working set (k-tiles 0-7,
                    # q columns 0-1024) first so the deep pipeline's ramp is
                    # never DMA-blocked, then stream the bulk
                    nc.sync.dma_start(out=vas[:, 0:8 * VW], in_=va_d[h][:, 0:8 * VW])
                    nc.sync.dma_start(out=qkts[:, 0:2 * QB], in_=qk_d[h][:, 0:2 * QB])
                    nc.sync.dma_start(
                        out=qkts[:, S:S + 2 * QB], in_=qk_d[h][:, S:S + 2 * QB]
                    )
                    nc.sync.dma_start(
                        out=vas[:, 8 * VW:], in_=va_d[h][:, 8 * VW:]
                    )
                    nc.sync.dma_start(out=qkts[:, 2 * QB:S], in_=qk_d[h][:, 2 * QB:S])
                    nc.sync.dma_start(
                        out=qkts[:, S + 2 * QB:2 * S], in_=qk_d[h][:, S + 2 * QB:2 * S]
                    )
                else:
                    nc.sync.dma_start(out=vas, in_=va_d[h])
                    # split halves onto separate DMA queues
                    nc.sync.dma_start(out=qkts[:, 0:S], in_=qk_d[h][:, 0:S])
                    nc.sync.dma_start(
                        out=qkts[:, S:2 * S], in_=qk_d[h][:, S:2 * S]
                    )
                head_ctx.append((vas, qkts, outs))

            # flat chunk list over (head, q-block): chunks of <=3 k-tiles;
            # one 3-bank PSUM tile + one exp (ScalarE or VectorE) per chunk
            def head_chunks(h, js):
                # chunks of exactly 2 k-tiles: one QK row-group pair, one
                # 2-bank stp tile -> stp can triple-buffer (3x2+2 = 8 banks)
                out = []
                for j in js:
                    n_kt = 4 * (j + 1)          # causal: k-tiles 0..4j+3
                    for k0 in range(0, n_kt, 2):
                        out.append((h, j, k0, 2, n_kt))
                return out

            # interleave head 0 (ascending j) with head 1 (descending j) so
            # the PE always has a large chunk in flight while the other
            # stream is in a small/diagonal region (smooths startup + drain)
            if os.environ.get("ATTN_ILV", "1") == "1":
                s0 = head_chunks(0, range(NQB))
                s1 = head_chunks(1, range(NQB - 1, -1, -1))
                all_chunks = []
                for a, b in zip(s0, s1):
                    all_chunks.append(a)
                    all_chunks.append(b)
            else:
                all_chunks = head_chunks(0, range(NQB)) + head_chunks(
                    1, range(NQB)
                )

            # engine routing: diagonal chunks -> ScalarE (exact exp);
            # off-diagonal chunks balance ScalarE/VectorE busy-time with a
            # preference for alternation (keeps both engines concurrently
            # busy within the software pipeline).
            # routing[idx] = list of (col_lo, col_hi, engine) exp pieces.
            # Diagonal chunks: pure ScalarE (exact exp where softmax rows are
            # short/peaked). Off-diagonal: first tile on one engine, rest on
            # the other, alternating; both engines then work the same chunk
            # concurrently, halving the exp latency ahead of the PV matmuls.
            eng_ns = {"act": 0.0, "dve": 0.0}

            dvec = float(os.environ.get("ATTN_DVEC", "145"))

            def exp_cost(eng, fd):
                return (fd + 222.0) / 1.2 if eng == "act" else (fd + dvec) / 0.96

            routing = []
            flip = False
            for (h, j, k0, clen, n_kt) in all_chunks:
                is_diag = (k0 + clen - 1) >= 4 * j
                pieces = []
                if is_diag:
                    pieces.append((0, clen * QB, "act"))
                    n_corner = sum(
                        1 for u in range(clen) if (k0 + u) - 4 * j >= 0
                    )
                    eng_ns["dve"] += n_corner * 260.0
                elif clen == 1:
                    eng = "act" if eng_ns["act"] <= eng_ns["dve"] else "dve"
                    pieces.append((0, QB, eng))
                else:
                    e1 = "dve" if flip else "act"
                    e2 = "act" if flip else "dve"
                    flip = not flip
                    if eng_ns[e1] > eng_ns[e2] + 4000.0:
                        e1, e2 = e2, e1
                    pieces.append((0, QB, e1))
                    pieces.append((QB, clen * QB, e2))
                for lo, hi, eng in pieces:
                    eng_ns[eng] += exp_cost(eng, hi - lo)
                if k0 + clen == n_kt:
                    eng_ns["dve"] += 754.0       # PSUM->SBUF out copy
                routing.append(pieces)

            otp_box = {}

            def emit_qks(idx):
                h, j, k0, clen, n_kt = all_chunks[idx]
                vas, qkts, outs = head_ctx[h]
                stp = stpool.tile([128, 2 * QB], F32, name="stp", tag="stp")
                # QK^T matmuls two-at-a-time in disjoint row groups
                # (rows 0-63 / 64-127 hold identical q,k data) so the PE
                # runs them concurrently. Diagonal tiles only produce
                # q-columns >= 128t (start=True bank-clear zeroes the rest).
                u = 0
                while u < clen:
                    for r in range(2 if u + 1 < clen else 1):
                        ki = k0 + u + r
                        t = ki - 4 * j
                        off = KT * t if t > 0 else 0
                        row = slice(r * D, (r + 1) * D)
                        mi = nc.tensor.matmul(
                            out=stp[:, (u + r) * QB + off:(u + r + 1) * QB],
                            lhsT=qkts[row, S + ki * KT:S + (ki + 1) * KT],
                            rhs=qkts[row, j * QB + off:(j + 1) * QB],
                            start=True,
                            stop=True,
                        )
                        _note(mi, "qk_diag" if t > 0 else "qk")
                    u += 2 if u + 1 < clen else 1
                pt = ppool.tile([128, 2 * QB], F16, name="pt", tag="pt")
                return stp, pt

            def emit_exp(idx, qk_pt):
                h, j, k0, clen, n_kt = all_chunks[idx]
                stp, pt = qk_pt
                # valid (written) column runs: diagonal tiles only produced
                # q-columns >= 128t, so merge per-tile valid ranges into
                # contiguous runs and exp only those (PSUM outside them is
                # uninitialized)
                runs = []
                for u in range(clen):
                    t = (k0 + u) - 4 * j
                    off = KT * t if t > 0 else 0
                    lo, hi = u * QB + off, (u + 1) * QB
                    if runs and runs[-1][1] == lo:
                        runs[-1][1] = hi
                    else:
                        runs.append([lo, hi])
                for plo, phi, eng in routing[idx]:
                    for rlo, rhi in runs:
                        lo, hi = max(plo, rlo), min(phi, rhi)
                        if lo >= hi:
                            continue
                        if eng == "act":
                            nc.scalar.activation(
                                out=pt[:, lo:hi], in_=stp[:, lo:hi],
                                func=mybir.ActivationFunctionType.Exp,
                                scale=1.0 / EXP_A,
                            )
                        else:
                            nc.vector.tensor_scalar(
                                out=pt[:, lo:hi].bitcast(I16),
                                in0=stp[:, lo:hi],
                                scalar1=SCHRAUDOLPH_B,
                                scalar2=None,
                                op0=mybir.AluOpType.add,
                            )

            def emit_masks(idx, qk_pt):
                # in-place 128x128 triangular corner masks (VectorE). Emitted
                # BEFORE the next chunk's VectorE convert so diagonal PVs are
                # not head-of-line blocked behind a 1.7us convert.
                h, j, k0, clen, n_kt = all_chunks[idx]
                stp, pt = qk_pt
                for u in range(clen):
                    t = (k0 + u) - 4 * j
                    if t >= 0:
                        cs = u * QB + KT * t
                        mask_eng = (
                            nc.gpsimd
                            if os.environ.get("ATTN_GMASK", "0") == "1"
                            else nc.vector
                        )
                        mask_eng.tensor_mul(
                            out=pt[:, cs:cs + KT],
                            in0=pt[:, cs:cs + KT],
                            in1=ctri,
                        )

            def emit_pvs(idx, qk_pt):
                h, j, k0, clen, n_kt = all_chunks[idx]
                stp, pt = qk_pt
                vas, qkts, outs = head_ctx[h]
                if (h, j) not in otp_box:
                    otp_box[(h, j)] = otpool.tile(
                        [VW, QB], F32, name="otp", tag="otp"
                    )
                otp = otp_box[(h, j)]
                for u in range(clen):
                    ki = k0 + u
                    t = ki - 4 * j
                    off = KT * t if t >= 0 else 0
                    mi = nc.tensor.matmul(
                        out=otp[:, off:QB],
                        lhsT=vas[:, ki * VW:(ki + 1) * VW],
                        rhs=pt[:, u * QB + off:(u + 1) * QB],
                        start=(ki == 0),
                        stop=(ki == n_kt - 1),
                    )
                    _note(mi, "pv_diag" if t >= 0 else "pv")
                if k0 + clen == n_kt:       # last chunk of this q-block
                    nc.vector.tensor_copy(
                        out=outs[:, j * QB:(j + 1) * QB], in_=otp
                    )
                    nc.sync.dma_start(
                        out=oT_d[h][:, j * QB:(j + 1) * QB],
                        in_=outs[:, j * QB:(j + 1) * QB],
                    )

            # 2-deep software pipeline. Per-engine FIFO orders per iteration:
            #   PE:  QK(i) ... PV(i-2)    (PV deps resolved ~2 chunks early)
            #   DVE: mask(i-1), conv(i)   (masks not HOL-blocked by convert)
            #   ACT: exp(i)
            hist = {}
            n_chunks = len(all_chunks)
            for idx in range(n_chunks):
                hist[idx] = emit_qks(idx)
                if idx >= 1:
                    emit_masks(idx - 1, hist[idx - 1])
                emit_exp(idx, hist[idx])
                if idx >= 2:
                    emit_pvs(idx - 2, hist.pop(idx - 2))
            emit_masks(n_chunks - 1, hist[n_chunks - 1])
            emit_pvs(n_chunks - 2, hist.pop(n_chunks - 2))
            emit_pvs(n_chunks - 1, hist.pop(n_chunks - 1))

    # TRN2 allows at most 1 semaphore wait per instruction (the fp32r
    # matmul's LDWEIGHTS slot enforces it); split surplus waits into
    # standalone EventSemaphore instructions like the bacc flow does.
    import concourse.bacc as baccmod

    baccmod._bass_rust.generate_event_semaphores(nc)
    return nc


_PROGRAM_CACHE: dict[str, bass.Bass] = {}


def get_program() -> bass.Bass:
    if "p" not in _PROGRAM_CACHE:
        _PROGRAM_CACHE["p"] = build_program()
    return _PROGRAM_CACHE["p"]


def make_corner_mask() -> np.ndarray:
    kk = np.arange(128)[:, None]
    qq = np.arange(KT)[None, :]
    return np.ascontiguousarray((qq >= kk).astype(np.float16))


def make_in_maps(q, k, v):
    q = np.asarray(q, dtype=np.float32)
    k = np.asarray(k, dtype=np.float32)
    v = np.asarray(v, dtype=np.float32)
    mk = make_corner_mask()
    in_maps = []
    for c in range(N_CORES):
        hs = [H_PER * c + i for i in range(H_PER)]
        qk = np.empty((H_PER, 2 * D, 2 * S), dtype=np.float16)
        va = np.empty((H_PER, 128, NKT, VW), dtype=np.float16)
        for i, h in enumerate(hs):
            qk[i, 0:D, 0:S] = q[0, h].T * QK_SIDE_SCALE
            qk[i, 0:D, S:2 * S] = k[0, h].T * QK_SIDE_SCALE
            qk[i, D:2 * D, :] = qk[i, 0:D, :]
            # [S, D] -> k-tiles on partitions: [128, NKT, D]
            va[i, :, :, :D] = v[0, h].reshape(NKT, KT, D).transpose(1, 0, 2)
            va[i, :, :, D] = 1.0
        in_maps.append(
            {
                "qk": qk,
                "va": np.ascontiguousarray(va.reshape(H_PER, 128, NKT * VW)),
                "mk": mk,
            }
        )
    return in_maps


def assemble_output(results) -> np.ndarray:
    out = np.empty((B, H, S, D), dtype=np.float32)
    for c in range(N_CORES):
        oT = results[c]["outT"]  # [H_PER, VW, S]
        for i in range(H_PER):
            h = H_PER * c + i
            out[0, h] = (oT[i, :D, :] / oT[i, D:D + 1, :]).T
    return out


def run_sharded(q, k, v, trace: bool = False):
    from concourse.bass_utils import run_bass_kernel_spmd

    nc = get_program()
    in_maps = make_in_maps(q, k, v)
    res = run_bass_kernel_spmd(
        nc, in_maps, list(range(N_CORES)), trace=trace
    )
    return assemble_output(res.results), res


def kernel(q, k, v, mask=None) -> np.ndarray:
    # mask is deterministically the causal tril mask; causality is baked in.
    out, _ = run_sharded(q, k, v, trace=False)
    return out
